# revision 18
# baseline (speedup 1.0000x reference)
"""MultiHeadEMA on 8 Trainium2 NeuronCores.

Strategy
--------
Channel-sharded: embed_dim=1024 -> 8 slices of 128 channels (= SBUF
partitions), one per core. The reference's FFT conv is exactly an order-2 IIR
    y_n[l] = q_n y_n[l-1] + x[l],   out = silu(c0 y0 + c1 y1 + omega x)
computed with `tensor_tensor_scan` on the vector engine.

The DVE scan runs at ~2.1 cyc/elem, so the recurrence is decimated by 4:
    Y_n[j] = y_n[4j] satisfies  Y_n[j] = q_n^4 Y_n[j-1] + u_n[j]
    u_n[j] = x[4j] + q_n x[4j-1] + q_n^2 x[4j-2] + q_n^3 x[4j-3]
u_n is built by accumulating diagonal matmuls (tensor engine, bf16) into
PSUM from contiguous phase blocks of x (deinterleaved and pre-shifted on the
host — a strided matmul rhs halves PE throughput). The scan reads u straight
from PSUM at 1/4 length. Phases y[4j+r] are never materialized: the outputs
    pre_r = c0 y0[4j+r] + c1 y1[4j+r] + w x[4j+r]
expand into diagonal matmuls over (Y0, Y1, phase blocks of x) with
per-channel coefficients (c_n q_n^r, sums), accumulated in PSUM, then one
Silu per 1024 columns evacuates PSUM -> SBUF (phase-major output, host
re-interleaves). Interior is bf16 (fp32 PSUM accumulation, fp32 scan state,
exact fp32 decay factors).

Engine balance at 8 concurrent cores: the chip power governor caps matmuls
at ~379ns (vs 216ns single-core) and punishes load added to the vector
engine (scans degrade), so the design keeps DVE scan-only and feeds the
tensor engine one dense contiguous stream.
"""

import numpy as np
import ml_dtypes

import concourse.bass as bass
import concourse.bacc as bacc
import concourse.tile as tile
from concourse import mybir
from concourse.bass_utils import run_bass_kernel_spmd

SEQ_LEN, BSZ, EMBED_DIM, NDIM = 4096, 4, 1024, 2
N_CORES = 8
D_PER = EMBED_DIM // N_CORES  # 128 channels/core = full SBUF partitions
SCALE = (1.0 / NDIM) ** 0.5
DEC = 4                   # decimation factor
J = SEQ_LEN // DEC        # decimated length 1024
CH = 512                  # matmul chunk (one fp32 PSUM bank)
NG = J // CH              # j-groups per slab (2)
F32 = mybir.dt.float32
BF16 = mybir.dt.bfloat16
AF = mybir.ActivationFunctionType
ALU = mybir.AluOpType

# x phase blocks, ordered so the u-generation inputs come first:
#   block 0..3: x[4j], x[4(j-1)+3], x[4(j-1)+2], x[4(j-1)+1]   (u inputs)
#   block 4..6: x[4j+1], x[4j+2], x[4j+3]                      (combine inputs)
NBLK = 7


def build_bass():
    nc = bacc.Bacc(name="multihead_ema")
    x = nc.dram_tensor("x", [D_PER, BSZ, NBLK, J], BF16, kind="ExternalInput")
    # coef columns: [delta0, delta1, alpha0, alpha1, beta0, beta1, gamma0, gamma1, omega]
    coef = nc.dram_tensor("coef", [D_PER, 9], F32, kind="ExternalInput")
    eye = nc.dram_tensor("eye", [D_PER, D_PER], BF16, kind="ExternalInput")
    out = nc.dram_tensor("out", [D_PER, BSZ, DEC, J], BF16, kind="ExternalOutput")

    with tile.TileContext(nc) as tc:
        with (
            tc.tile_pool(name="const", bufs=1) as const,
            tc.tile_pool(name="xup", bufs=4) as xup,
            tc.tile_pool(name="xcp", bufs=4) as xcp,
            tc.tile_pool(name="yp", bufs=2) as yp,
            tc.tile_pool(name="op", bufs=3) as op,
            tc.tile_pool(name="psu", bufs=2, space="PSUM") as psu,
            tc.tile_pool(name="psc", bufs=2, space="PSUM") as psc,
        ):
            csb = const.tile([D_PER, 9], F32)
            nc.sync.dma_start(out=csb[:, :], in_=coef[:, :])
            eyesb = const.tile([D_PER, D_PER], BF16)
            nc.sync.dma_start(out=eyesb[:, :], in_=eye[:, :])

            # --- per-channel coefficients ([128, 1/2] fp32, trivial)
            sig = const.tile([D_PER, 4], F32)  # [p0, p1, sa0, sa1]
            nc.scalar.activation(out=sig[:, :], in_=csb[:, 0:4], func=AF.Sigmoid)
            pq = const.tile([D_PER, NDIM], F32)
            nc.vector.tensor_mul(out=pq[:, :], in0=sig[:, 0:2], in1=sig[:, 2:4])
            q = const.tile([D_PER, NDIM], F32)  # q = 1 - p*sigmoid(alpha)
            nc.scalar.activation(out=q[:, :], in_=pq[:, :], func=AF.Copy,
                                 scale=-1.0, bias=1.0)
            q2 = const.tile([D_PER, NDIM], F32)
            nc.vector.tensor_mul(out=q2[:, :], in0=q[:, :], in1=q[:, :])
            q3 = const.tile([D_PER, NDIM], F32)
            nc.vector.tensor_mul(out=q3[:, :], in0=q2[:, :], in1=q[:, :])
            q4 = const.tile([D_PER, NDIM], F32)
            nc.vector.tensor_mul(out=q4[:, :], in0=q2[:, :], in1=q2[:, :])
            c1t = const.tile([D_PER, NDIM], F32)
            nc.vector.tensor_mul(out=c1t[:, :], in0=sig[:, 0:2], in1=csb[:, 4:6])
            c2t = const.tile([D_PER, NDIM], F32)
            nc.vector.tensor_mul(out=c2t[:, :], in0=c1t[:, :], in1=csb[:, 6:8])
            cc = const.tile([D_PER, NDIM], F32)  # c_n = p beta gamma scale
            nc.scalar.mul(out=cc[:, :], in_=c2t[:, :], mul=SCALE)
            cq = const.tile([D_PER, NDIM], F32)   # c_n q_n
            nc.vector.tensor_mul(out=cq[:, :], in0=cc[:, :], in1=q[:, :])
            cq2 = const.tile([D_PER, NDIM], F32)  # c_n q_n^2
            nc.vector.tensor_mul(out=cq2[:, :], in0=cc[:, :], in1=q2[:, :])
            cq3 = const.tile([D_PER, NDIM], F32)  # c_n q_n^3
            nc.vector.tensor_mul(out=cq3[:, :], in0=cc[:, :], in1=q3[:, :])
            csum = const.tile([D_PER, 1], F32)    # c0 + c1 + w
            nc.vector.tensor_add(out=csum[:, :], in0=cc[:, 0:1], in1=cc[:, 1:2])
            nc.vector.tensor_add(out=csum[:, :], in0=csum[:, :], in1=csb[:, 8:9])
            cqs = const.tile([D_PER, 1], F32)     # c0 q0 + c1 q1
            nc.vector.tensor_add(out=cqs[:, :], in0=cq[:, 0:1], in1=cq[:, 1:2])
            cq2s = const.tile([D_PER, 1], F32)    # c0 q0^2 + c1 q1^2
            nc.vector.tensor_add(out=cq2s[:, :], in0=cq2[:, 0:1], in1=cq2[:, 1:2])

            # --- bf16 diagonal weight matrices
            _dn = [0]

            def diag(scalar_ap):
                _dn[0] += 1
                t = const.tile([D_PER, D_PER], BF16, tag=f"diag{_dn[0]}")
                nc.vector.tensor_scalar_mul(out=t[:, :], in0=eyesb[:, :],
                                            scalar1=scalar_ap)
                return t

            w_q = [[diag(t[:, n : n + 1]) for n in range(NDIM)] for t in (q, q2, q3)]
            w_cy = [[diag(t[:, n : n + 1]) for n in range(NDIM)]
                    for t in (cc, cq, cq2, cq3)]  # Y-term weights for r=0..3
            w_w = diag(csb[:, 8:9])    # x term of pre_0
            w_cw = diag(csum[:, 0:1])  # x_pr self term, r>=1
            w_cqs = diag(cqs[:, 0:1])
            w_cq2s = diag(cq2s[:, 0:1])

            q4b = [q4[:, n : n + 1].to_broadcast([D_PER, J]) for n in range(NDIM)]

            for b in range(BSZ):
                xall = xup.tile([D_PER, NBLK, J], BF16)
                nc.sync.dma_start(out=xall[:, :, :], in_=x[:, b, :, :])
                xu = xall[:, 0:4, :]
                xc = xall[:, 4:7, :]

                # --- u_n in PSUM, Y_n = scan(q_n^4, u_n)
                Y = []
                for n in range(NDIM):
                    pu = psu.tile([D_PER, J], F32, tag="u")
                    for g in range(NG):
                        s = bass.ts(g, CH)
                        nc.tensor.matmul(pu[:, s], eyesb[:, :], xu[:, 0, s],
                                         start=True, stop=False)
                        for k in range(1, 4):  # + q^k * x[4j-k] (pre-shifted block k)
                            nc.tensor.matmul(pu[:, s], w_q[k - 1][n][:, :],
                                             xu[:, k, s],
                                             start=False, stop=(k == 3))
                    yn = yp.tile([D_PER, J], BF16, tag=f"y{n}")
                    nc.vector.tensor_tensor_scan(
                        out=yn[:, :], data0=q4b[n], data1=pu[:, :],
                        initial=0.0, op0=ALU.mult, op1=ALU.add,
                    )
                    Y.append(yn)

                # --- outputs: pre_r accumulated in PSUM, silu evacuates
                ob = op.tile([D_PER, DEC, J], BF16)
                for g in range(NG):
                    s = bass.ts(g, CH)
                    for pair in range(2):  # phases (0,1) then (2,3)
                        pt = psc.tile([D_PER, 2 * CH], F32, tag="cmb")
                        for h in range(2):
                            r = 2 * pair + h
                            tgt = pt[:, bass.ts(h, CH)]
                            nc.tensor.matmul(tgt, w_cy[r][0][:, :], Y[0][:, s],
                                             start=True, stop=False)
                            nc.tensor.matmul(tgt, w_cy[r][1][:, :], Y[1][:, s],
                                             start=False, stop=False)
                            # x terms: phase r block is xc[r-1] (r>=1), xu[0] for r=0
                            xw = [(w_w, None) if r == 0 else (w_cw, r)]
                            if r == 2:
                                xw.append((w_cqs, 1))
                            elif r == 3:
                                xw.append((w_cqs, 2))
                                xw.append((w_cq2s, 1))
                            for i, (wt, rr) in enumerate(xw):
                                rhs = xu[:, 0, s] if rr is None else xc[:, rr - 1, s]
                                nc.tensor.matmul(tgt, wt[:, :], rhs,
                                                 start=False, stop=(i == len(xw) - 1))
                        # silu: pt[:, h*512 + k] -> ob[:, 2*pair + h, 512g + k]
                        in_ap = pt[:, :].rearrange("p (h k) -> p h k", h=2)
                        nc.scalar.activation(
                            out=ob[:, 2 * pair : 2 * pair + 2, s],
                            in_=in_ap, func=AF.Silu)
                nc.sync.dma_start(out=out[:, b, :, :], in_=ob[:, :, :])

    nc.compile()
    return nc


_CACHE: dict = {}


def _get_nc():
    if "nc" not in _CACHE:
        _CACHE["nc"] = build_bass()
    return _CACHE["nc"]


def make_in_maps(inputs):
    x = np.asarray(inputs["x"], np.float32)
    delta = np.asarray(inputs["delta"], np.float32).reshape(EMBED_DIM, NDIM)
    alpha = np.asarray(inputs["alpha"], np.float32).reshape(EMBED_DIM, NDIM)
    beta = np.asarray(inputs["beta"], np.float32).reshape(EMBED_DIM, NDIM)
    gamma = np.asarray(inputs["gamma"], np.float32).reshape(EMBED_DIM, NDIM)
    omega = np.asarray(inputs["omega"], np.float32).reshape(EMBED_DIM, 1)
    coef_full = np.concatenate([delta, alpha, beta, gamma, omega], axis=1)
    eye = np.eye(D_PER, dtype=ml_dtypes.bfloat16)
    in_maps = []
    for c in range(N_CORES):
        sl = slice(c * D_PER, (c + 1) * D_PER)
        xc = x[:, :, sl].transpose(2, 1, 0).astype(ml_dtypes.bfloat16)  # [128,B,L]
        ph = xc.reshape(D_PER, BSZ, J, DEC).transpose(0, 1, 3, 2)  # [128,B,4,J]
        xph = np.zeros((D_PER, BSZ, NBLK, J), dtype=ml_dtypes.bfloat16)
        xph[:, :, 0] = ph[:, :, 0]                    # x[4j]
        for k in range(1, 4):                         # x[4(j-1) + (4-k)] = x[4j-k]
            xph[:, :, k, 1:] = ph[:, :, 4 - k, :-1]
        xph[:, :, 4:7] = ph[:, :, 1:4]                # x[4j+r], r=1..3
        in_maps.append(
            {"x": np.ascontiguousarray(xph),
             "coef": np.ascontiguousarray(coef_full[sl]), "eye": eye}
        )
    return in_maps


def gather_out(results):
    out = np.empty((SEQ_LEN, BSZ, EMBED_DIM), np.float32)
    for c in range(N_CORES):
        # [128, B, 4, J] phase-major -> [l = 4j+r, b, d]
        arr = results[c]["out"].astype(np.float32)
        out[:, :, c * D_PER : (c + 1) * D_PER] = arr.transpose(3, 2, 1, 0).reshape(
            SEQ_LEN, BSZ, D_PER
        )
    return out


def _run(inputs, **kwargs):
    nc = _get_nc()
    in_maps = make_in_maps(inputs)
    res = run_bass_kernel_spmd(nc, in_maps, core_ids=list(range(N_CORES)), **kwargs)
    return gather_out(res.results), res


def kernel(**inputs) -> np.ndarray:
    out, _ = _run(inputs)
    return out


# revision 19
# speedup vs baseline: 1.0951x; 1.0951x over previous
"""MultiHeadEMA on 8 Trainium2 NeuronCores.

Strategy
--------
Channel-sharded: embed_dim=1024 -> 8 slices of 128 channels (= SBUF
partitions), one per core. The reference's FFT conv is exactly an order-2 IIR
    y_n[l] = q_n y_n[l-1] + x[l],   out = silu(c0 y0 + c1 y1 + omega x)
computed with `tensor_tensor_scan` on the vector engine.

The DVE scan runs at ~2.1 cyc/elem, so the recurrence is decimated by 4:
    Y_n[j] = y_n[4j] satisfies  Y_n[j] = q_n^4 Y_n[j-1] + u_n[j]
    u_n[j] = x[4j] + q_n x[4j-1] + q_n^2 x[4j-2] + q_n^3 x[4j-3]
u_n is built by accumulating diagonal matmuls (tensor engine, bf16) into
PSUM from contiguous phase blocks of x (deinterleaved and pre-shifted on the
host — a strided matmul rhs halves PE throughput). The scan reads u straight
from PSUM at 1/4 length. Phases y[4j+r] are never materialized: the outputs
    pre_r = c0 y0[4j+r] + c1 y1[4j+r] + w x[4j+r]
expand into diagonal matmuls over (Y0, Y1, phase blocks of x) with
per-channel coefficients (c_n q_n^r, sums), accumulated in PSUM, then one
Silu per 1024 columns evacuates PSUM -> SBUF (phase-major output, host
re-interleaves). Interior is bf16 (fp32 PSUM accumulation, fp32 scan state,
exact fp32 decay factors).

Engine balance at 8 concurrent cores: the chip power governor caps matmuls
at ~379ns (vs 216ns single-core) and punishes load added to the vector
engine (scans degrade), so the design keeps DVE scan-only and feeds the
tensor engine one dense contiguous stream.
"""

import numpy as np
import ml_dtypes

import concourse.bass as bass
import concourse.bacc as bacc
import concourse.tile as tile
from concourse import mybir
from concourse.bass_utils import run_bass_kernel_spmd

SEQ_LEN, BSZ, EMBED_DIM, NDIM = 4096, 4, 1024, 2
N_CORES = 8
D_PER = EMBED_DIM // N_CORES  # 128 channels/core = full SBUF partitions
SCALE = (1.0 / NDIM) ** 0.5
DEC = 4                   # decimation factor
J = SEQ_LEN // DEC        # decimated length 1024
CH = 512                  # matmul chunk (one fp32 PSUM bank)
NG = J // CH              # j-groups per slab (2)
F32 = mybir.dt.float32
BF16 = mybir.dt.bfloat16
AF = mybir.ActivationFunctionType
ALU = mybir.AluOpType

# x phase blocks, ordered so the u-generation inputs come first:
#   block 0..3: x[4j], x[4(j-1)+3], x[4(j-1)+2], x[4(j-1)+1]   (u inputs)
#   block 4..6: x[4j+1], x[4j+2], x[4j+3]                      (combine inputs)
NBLK = 7


def build_bass():
    nc = bacc.Bacc(name="multihead_ema")
    x = nc.dram_tensor("x", [D_PER, BSZ, NBLK, J], BF16, kind="ExternalInput")
    # coef columns: [delta0, delta1, alpha0, alpha1, beta0, beta1, gamma0, gamma1, omega]
    coef = nc.dram_tensor("coef", [D_PER, 9], F32, kind="ExternalInput")
    eye = nc.dram_tensor("eye", [D_PER, D_PER], BF16, kind="ExternalInput")
    out = nc.dram_tensor("out", [D_PER, BSZ, DEC, J], BF16, kind="ExternalOutput")

    with tile.TileContext(nc) as tc:
        with (
            tc.tile_pool(name="const", bufs=1) as const,
            tc.tile_pool(name="xup", bufs=4) as xup,
            tc.tile_pool(name="xcp", bufs=4) as xcp,
            tc.tile_pool(name="yp", bufs=2) as yp,
            tc.tile_pool(name="op", bufs=3) as op,
            tc.tile_pool(name="psu", bufs=2, space="PSUM") as psu,
            tc.tile_pool(name="psc", bufs=2, space="PSUM") as psc,
        ):
            csb = const.tile([D_PER, 9], F32)
            nc.sync.dma_start(out=csb[:, :], in_=coef[:, :])
            eyesb = const.tile([D_PER, D_PER], BF16)
            nc.sync.dma_start(out=eyesb[:, :], in_=eye[:, :])

            # --- per-channel coefficients ([128, 1/2] fp32, trivial)
            sig = const.tile([D_PER, 4], F32)  # [p0, p1, sa0, sa1]
            nc.scalar.activation(out=sig[:, :], in_=csb[:, 0:4], func=AF.Sigmoid)
            pq = const.tile([D_PER, NDIM], F32)
            nc.vector.tensor_mul(out=pq[:, :], in0=sig[:, 0:2], in1=sig[:, 2:4])
            q = const.tile([D_PER, NDIM], F32)  # q = 1 - p*sigmoid(alpha)
            nc.scalar.activation(out=q[:, :], in_=pq[:, :], func=AF.Copy,
                                 scale=-1.0, bias=1.0)
            q2 = const.tile([D_PER, NDIM], F32)
            nc.vector.tensor_mul(out=q2[:, :], in0=q[:, :], in1=q[:, :])
            q3 = const.tile([D_PER, NDIM], F32)
            nc.vector.tensor_mul(out=q3[:, :], in0=q2[:, :], in1=q[:, :])
            q4 = const.tile([D_PER, NDIM], F32)
            nc.vector.tensor_mul(out=q4[:, :], in0=q2[:, :], in1=q2[:, :])
            c1t = const.tile([D_PER, NDIM], F32)
            nc.vector.tensor_mul(out=c1t[:, :], in0=sig[:, 0:2], in1=csb[:, 4:6])
            c2t = const.tile([D_PER, NDIM], F32)
            nc.vector.tensor_mul(out=c2t[:, :], in0=c1t[:, :], in1=csb[:, 6:8])
            cc = const.tile([D_PER, NDIM], F32)  # c_n = p beta gamma scale
            nc.scalar.mul(out=cc[:, :], in_=c2t[:, :], mul=SCALE)
            cq = const.tile([D_PER, NDIM], F32)   # c_n q_n
            nc.vector.tensor_mul(out=cq[:, :], in0=cc[:, :], in1=q[:, :])
            cq2 = const.tile([D_PER, NDIM], F32)  # c_n q_n^2
            nc.vector.tensor_mul(out=cq2[:, :], in0=cc[:, :], in1=q2[:, :])
            cq3 = const.tile([D_PER, NDIM], F32)  # c_n q_n^3
            nc.vector.tensor_mul(out=cq3[:, :], in0=cc[:, :], in1=q3[:, :])
            csum = const.tile([D_PER, 1], F32)    # c0 + c1 + w
            nc.vector.tensor_add(out=csum[:, :], in0=cc[:, 0:1], in1=cc[:, 1:2])
            nc.vector.tensor_add(out=csum[:, :], in0=csum[:, :], in1=csb[:, 8:9])
            cqs = const.tile([D_PER, 1], F32)     # c0 q0 + c1 q1
            nc.vector.tensor_add(out=cqs[:, :], in0=cq[:, 0:1], in1=cq[:, 1:2])
            cq2s = const.tile([D_PER, 1], F32)    # c0 q0^2 + c1 q1^2
            nc.vector.tensor_add(out=cq2s[:, :], in0=cq2[:, 0:1], in1=cq2[:, 1:2])

            # --- bf16 diagonal weight matrices
            _dn = [0]

            def diag(scalar_ap):
                _dn[0] += 1
                t = const.tile([D_PER, D_PER], BF16, tag=f"diag{_dn[0]}")
                nc.vector.tensor_scalar_mul(out=t[:, :], in0=eyesb[:, :],
                                            scalar1=scalar_ap)
                return t

            w_q = [[diag(t[:, n : n + 1]) for n in range(NDIM)] for t in (q, q2, q3)]
            w_cy = [[diag(t[:, n : n + 1]) for n in range(NDIM)]
                    for t in (cc, cq, cq2, cq3)]  # Y-term weights for r=0..3
            w_w = diag(csb[:, 8:9])    # x term of pre_0
            w_cw = diag(csum[:, 0:1])  # x_pr self term, r>=1
            w_cqs = diag(cqs[:, 0:1])
            w_cq2s = diag(cq2s[:, 0:1])

            q4b = [q4[:, n : n + 1].to_broadcast([D_PER, J]) for n in range(NDIM)]

            for b in range(BSZ):
                xall = xup.tile([D_PER, NBLK, J], BF16)
                nc.sync.dma_start(out=xall[:, :, :], in_=x[:, b, :, :])
                xu = xall[:, 0:4, :]
                xc = xall[:, 4:7, :]

                # --- u_n in PSUM, Y_n = scan(q_n^4, u_n)
                Y = []
                for n in range(NDIM):
                    pu = psu.tile([D_PER, J], F32, tag="u")
                    # x[4j] term: DVE copy into PSUM (frees the tensor engine)
                    nc.vector.tensor_copy(out=pu[:, :], in_=xu[:, 0, :])
                    for g in range(NG):
                        s = bass.ts(g, CH)
                        for k in range(1, 4):  # + q^k * x[4j-k] (pre-shifted block k)
                            nc.tensor.matmul(pu[:, s], w_q[k - 1][n][:, :],
                                             xu[:, k, s],
                                             start=False, stop=(k == 3))
                    yn = yp.tile([D_PER, J], BF16, tag=f"y{n}")
                    nc.vector.tensor_tensor_scan(
                        out=yn[:, :], data0=q4b[n], data1=pu[:, :],
                        initial=0.0, op0=ALU.mult, op1=ALU.add,
                    )
                    Y.append(yn)

                # --- outputs: pre_r accumulated in PSUM, silu evacuates
                ob = op.tile([D_PER, DEC, J], BF16)
                for g in range(NG):
                    s = bass.ts(g, CH)
                    for pair in range(2):  # phases (0,1) then (2,3)
                        pt = psc.tile([D_PER, 2 * CH], F32, tag="cmb")
                        for h in range(2):
                            r = 2 * pair + h
                            tgt = pt[:, bass.ts(h, CH)]
                            nc.tensor.matmul(tgt, w_cy[r][0][:, :], Y[0][:, s],
                                             start=True, stop=False)
                            nc.tensor.matmul(tgt, w_cy[r][1][:, :], Y[1][:, s],
                                             start=False, stop=False)
                            # x terms: phase r block is xc[r-1] (r>=1), xu[0] for r=0
                            xw = [(w_w, None) if r == 0 else (w_cw, r)]
                            if r == 2:
                                xw.append((w_cqs, 1))
                            elif r == 3:
                                xw.append((w_cqs, 2))
                                xw.append((w_cq2s, 1))
                            for i, (wt, rr) in enumerate(xw):
                                rhs = xu[:, 0, s] if rr is None else xc[:, rr - 1, s]
                                nc.tensor.matmul(tgt, wt[:, :], rhs,
                                                 start=False, stop=(i == len(xw) - 1))
                        # silu: pt[:, h*512 + k] -> ob[:, 2*pair + h, 512g + k]
                        in_ap = pt[:, :].rearrange("p (h k) -> p h k", h=2)
                        nc.scalar.activation(
                            out=ob[:, 2 * pair : 2 * pair + 2, s],
                            in_=in_ap, func=AF.Silu)
                nc.sync.dma_start(out=out[:, b, :, :], in_=ob[:, :, :])

    nc.compile()
    return nc


_CACHE: dict = {}


def _get_nc():
    if "nc" not in _CACHE:
        _CACHE["nc"] = build_bass()
    return _CACHE["nc"]


def make_in_maps(inputs):
    x = np.asarray(inputs["x"], np.float32)
    delta = np.asarray(inputs["delta"], np.float32).reshape(EMBED_DIM, NDIM)
    alpha = np.asarray(inputs["alpha"], np.float32).reshape(EMBED_DIM, NDIM)
    beta = np.asarray(inputs["beta"], np.float32).reshape(EMBED_DIM, NDIM)
    gamma = np.asarray(inputs["gamma"], np.float32).reshape(EMBED_DIM, NDIM)
    omega = np.asarray(inputs["omega"], np.float32).reshape(EMBED_DIM, 1)
    coef_full = np.concatenate([delta, alpha, beta, gamma, omega], axis=1)
    eye = np.eye(D_PER, dtype=ml_dtypes.bfloat16)
    in_maps = []
    for c in range(N_CORES):
        sl = slice(c * D_PER, (c + 1) * D_PER)
        xc = x[:, :, sl].transpose(2, 1, 0).astype(ml_dtypes.bfloat16)  # [128,B,L]
        ph = xc.reshape(D_PER, BSZ, J, DEC).transpose(0, 1, 3, 2)  # [128,B,4,J]
        xph = np.zeros((D_PER, BSZ, NBLK, J), dtype=ml_dtypes.bfloat16)
        xph[:, :, 0] = ph[:, :, 0]                    # x[4j]
        for k in range(1, 4):                         # x[4(j-1) + (4-k)] = x[4j-k]
            xph[:, :, k, 1:] = ph[:, :, 4 - k, :-1]
        xph[:, :, 4:7] = ph[:, :, 1:4]                # x[4j+r], r=1..3
        in_maps.append(
            {"x": np.ascontiguousarray(xph),
             "coef": np.ascontiguousarray(coef_full[sl]), "eye": eye}
        )
    return in_maps


def gather_out(results):
    out = np.empty((SEQ_LEN, BSZ, EMBED_DIM), np.float32)
    for c in range(N_CORES):
        # [128, B, 4, J] phase-major -> [l = 4j+r, b, d]
        arr = results[c]["out"].astype(np.float32)
        out[:, :, c * D_PER : (c + 1) * D_PER] = arr.transpose(3, 2, 1, 0).reshape(
            SEQ_LEN, BSZ, D_PER
        )
    return out


def _run(inputs, **kwargs):
    nc = _get_nc()
    in_maps = make_in_maps(inputs)
    res = run_bass_kernel_spmd(nc, in_maps, core_ids=list(range(N_CORES)), **kwargs)
    return gather_out(res.results), res


def kernel(**inputs) -> np.ndarray:
    out, _ = _run(inputs)
    return out


# revision 21
# speedup vs baseline: 1.1650x; 1.0639x over previous
"""MultiHeadEMA on 8 Trainium2 NeuronCores.

Strategy
--------
Channel-sharded: embed_dim=1024 -> 8 slices of 128 channels (= SBUF
partitions), one per core. The reference's FFT conv is exactly an order-2 IIR
    y_n[l] = q_n y_n[l-1] + x[l],   out = silu(c0 y0 + c1 y1 + omega x)
computed with `tensor_tensor_scan` on the vector engine.

The DVE scan runs at ~2.1 cyc/elem, so the recurrence is decimated by 4:
    Y_n[j] = y_n[4j] satisfies  Y_n[j] = q_n^4 Y_n[j-1] + u_n[j]
    u_n[j] = x[4j] + q_n x[4j-1] + q_n^2 x[4j-2] + q_n^3 x[4j-3]
u_n is built by accumulating diagonal matmuls (tensor engine, bf16) into
PSUM from contiguous phase blocks of x (deinterleaved and pre-shifted on the
host — a strided matmul rhs halves PE throughput). The scan reads u straight
from PSUM at 1/4 length. Phases y[4j+r] are never materialized: the outputs
    pre_r = c0 y0[4j+r] + c1 y1[4j+r] + w x[4j+r]
expand into diagonal matmuls over (Y0, Y1, phase blocks of x) with
per-channel coefficients (c_n q_n^r, sums), accumulated in PSUM, then one
Silu per 1024 columns evacuates PSUM -> SBUF (phase-major output, host
re-interleaves). Interior is bf16 (fp32 PSUM accumulation, fp32 scan state,
exact fp32 decay factors).

Engine balance at 8 concurrent cores: the chip power governor caps matmuls
at ~379ns (vs 216ns single-core) and punishes load added to the vector
engine (scans degrade), so the design keeps DVE scan-only and feeds the
tensor engine one dense contiguous stream.
"""

import numpy as np
import ml_dtypes

import concourse.bass as bass
import concourse.bacc as bacc
import concourse.tile as tile
from concourse import mybir
from concourse.bass_utils import run_bass_kernel_spmd

SEQ_LEN, BSZ, EMBED_DIM, NDIM = 4096, 4, 1024, 2
N_CORES = 8
D_PER = EMBED_DIM // N_CORES  # 128 channels/core = full SBUF partitions
SCALE = (1.0 / NDIM) ** 0.5
DEC = 4                   # decimation factor
J = SEQ_LEN // DEC        # decimated length 1024
CH = 512                  # matmul chunk (one fp32 PSUM bank)
NG = J // CH              # j-groups per slab (2)
F32 = mybir.dt.float32
BF16 = mybir.dt.bfloat16
AF = mybir.ActivationFunctionType
ALU = mybir.AluOpType

# x phase blocks, ordered so the u-generation inputs come first:
#   block 0..3: x[4j], x[4(j-1)+3], x[4(j-1)+2], x[4(j-1)+1]   (u inputs)
#   block 4..6: x[4j+1], x[4j+2], x[4j+3]                      (combine inputs)
NBLK = 7


def build_bass():
    nc = bacc.Bacc(name="multihead_ema")
    x = nc.dram_tensor("x", [D_PER, BSZ, NBLK, J], BF16, kind="ExternalInput")
    # coef columns: [delta0, delta1, alpha0, alpha1, beta0, beta1, gamma0, gamma1, omega]
    coef = nc.dram_tensor("coef", [D_PER, 9], F32, kind="ExternalInput")
    eye = nc.dram_tensor("eye", [D_PER, D_PER], BF16, kind="ExternalInput")
    out = nc.dram_tensor("out", [D_PER, BSZ, DEC, J], BF16, kind="ExternalOutput")

    with tile.TileContext(nc) as tc:
        with (
            tc.tile_pool(name="const", bufs=1) as const,
            tc.tile_pool(name="xup", bufs=4) as xup,
            tc.tile_pool(name="xcp", bufs=4) as xcp,
            tc.tile_pool(name="yp", bufs=2) as yp,
            tc.tile_pool(name="op", bufs=3) as op,
            tc.tile_pool(name="psu", bufs=2, space="PSUM") as psu,
            tc.tile_pool(name="psc", bufs=2, space="PSUM") as psc,
        ):
            csb = const.tile([D_PER, 9], F32)
            nc.sync.dma_start(out=csb[:, :], in_=coef[:, :])
            eyesb = const.tile([D_PER, D_PER], BF16)
            nc.sync.dma_start(out=eyesb[:, :], in_=eye[:, :])

            # --- per-channel coefficients ([128, 1/2] fp32, trivial)
            sig = const.tile([D_PER, 4], F32)  # [p0, p1, sa0, sa1]
            nc.scalar.activation(out=sig[:, :], in_=csb[:, 0:4], func=AF.Sigmoid)
            pq = const.tile([D_PER, NDIM], F32)
            nc.vector.tensor_mul(out=pq[:, :], in0=sig[:, 0:2], in1=sig[:, 2:4])
            q = const.tile([D_PER, NDIM], F32)  # q = 1 - p*sigmoid(alpha)
            nc.scalar.activation(out=q[:, :], in_=pq[:, :], func=AF.Copy,
                                 scale=-1.0, bias=1.0)
            q2 = const.tile([D_PER, NDIM], F32)
            nc.vector.tensor_mul(out=q2[:, :], in0=q[:, :], in1=q[:, :])
            q3 = const.tile([D_PER, NDIM], F32)
            nc.vector.tensor_mul(out=q3[:, :], in0=q2[:, :], in1=q[:, :])
            q4 = const.tile([D_PER, NDIM], F32)
            nc.vector.tensor_mul(out=q4[:, :], in0=q2[:, :], in1=q2[:, :])
            c1t = const.tile([D_PER, NDIM], F32)
            nc.vector.tensor_mul(out=c1t[:, :], in0=sig[:, 0:2], in1=csb[:, 4:6])
            c2t = const.tile([D_PER, NDIM], F32)
            nc.vector.tensor_mul(out=c2t[:, :], in0=c1t[:, :], in1=csb[:, 6:8])
            cc = const.tile([D_PER, NDIM], F32)  # c_n = p beta gamma scale
            nc.scalar.mul(out=cc[:, :], in_=c2t[:, :], mul=SCALE)
            cq = const.tile([D_PER, NDIM], F32)   # c_n q_n
            nc.vector.tensor_mul(out=cq[:, :], in0=cc[:, :], in1=q[:, :])
            cq2 = const.tile([D_PER, NDIM], F32)  # c_n q_n^2
            nc.vector.tensor_mul(out=cq2[:, :], in0=cc[:, :], in1=q2[:, :])
            cq3 = const.tile([D_PER, NDIM], F32)  # c_n q_n^3
            nc.vector.tensor_mul(out=cq3[:, :], in0=cc[:, :], in1=q3[:, :])
            csum = const.tile([D_PER, 1], F32)    # c0 + c1 + w
            nc.vector.tensor_add(out=csum[:, :], in0=cc[:, 0:1], in1=cc[:, 1:2])
            nc.vector.tensor_add(out=csum[:, :], in0=csum[:, :], in1=csb[:, 8:9])
            cqs = const.tile([D_PER, 1], F32)     # c0 q0 + c1 q1
            nc.vector.tensor_add(out=cqs[:, :], in0=cq[:, 0:1], in1=cq[:, 1:2])
            cq2s = const.tile([D_PER, 1], F32)    # c0 q0^2 + c1 q1^2
            nc.vector.tensor_add(out=cq2s[:, :], in0=cq2[:, 0:1], in1=cq2[:, 1:2])

            # --- bf16 diagonal weight matrices
            _dn = [0]

            def diag(scalar_ap):
                _dn[0] += 1
                t = const.tile([D_PER, D_PER], BF16, tag=f"diag{_dn[0]}")
                nc.vector.tensor_scalar_mul(out=t[:, :], in0=eyesb[:, :],
                                            scalar1=scalar_ap)
                return t

            w_q = [[diag(t[:, n : n + 1]) for n in range(NDIM)] for t in (q, q2, q3)]
            w_cy = [[diag(t[:, n : n + 1]) for n in range(NDIM)]
                    for t in (cc, cq, cq2, cq3)]  # Y-term weights for r=0..3
            w_w = diag(csb[:, 8:9])    # x term of pre_0
            w_cw = diag(csum[:, 0:1])  # x_pr self term, r>=1
            w_cqs = diag(cqs[:, 0:1])
            w_cq2s = diag(cq2s[:, 0:1])

            q4b = [q4[:, n : n + 1].to_broadcast([D_PER, J]) for n in range(NDIM)]

            for b in range(BSZ):
                xall = xup.tile([D_PER, NBLK, J], BF16)
                nc.sync.dma_start(out=xall[:, :, :], in_=x[:, b, :, :])
                xu = xall[:, 0:4, :]
                xc = xall[:, 4:7, :]

                # --- u_n in PSUM, Y_n = scan(q_n^4, u_n)
                Y = []
                for n in range(NDIM):
                    pu = psu.tile([D_PER, J], F32, tag="u")
                    # x[4j] term: DVE copy into PSUM (frees the tensor engine)
                    nc.vector.tensor_copy(out=pu[:, :], in_=xu[:, 0, :])
                    for g in range(NG):
                        s = bass.ts(g, CH)
                        for k in range(1, 4):  # + q^k * x[4j-k] (pre-shifted block k)
                            nc.tensor.matmul(pu[:, s], w_q[k - 1][n][:, :],
                                             xu[:, k, s],
                                             start=False, stop=(k == 3))
                    yn = yp.tile([D_PER, J], BF16, tag=f"y{n}")
                    nc.vector.tensor_tensor_scan(
                        out=yn[:, :], data0=q4b[n], data1=pu[:, :],
                        initial=0.0, op0=ALU.mult, op1=ALU.add,
                    )
                    Y.append(yn)

                # --- outputs: pre_r accumulated in PSUM, silu evacuates
                ob = op.tile([D_PER, DEC, J], BF16)
                for pair in range(2):  # phases (0,1) then (2,3)
                    for g in range(NG):
                        s = bass.ts(g, CH)
                        pt = psc.tile([D_PER, 2 * CH], F32, tag="cmb")
                        for h in range(2):
                            r = 2 * pair + h
                            tgt = pt[:, bass.ts(h, CH)]
                            nc.tensor.matmul(tgt, w_cy[r][0][:, :], Y[0][:, s],
                                             start=True, stop=False)
                            nc.tensor.matmul(tgt, w_cy[r][1][:, :], Y[1][:, s],
                                             start=False, stop=False)
                            # x terms: phase r block is xc[r-1] (r>=1), xu[0] for r=0
                            xw = [(w_w, None) if r == 0 else (w_cw, r)]
                            if r == 2:
                                xw.append((w_cqs, 1))
                            elif r == 3:
                                xw.append((w_cqs, 2))
                                xw.append((w_cq2s, 1))
                            for i, (wt, rr) in enumerate(xw):
                                rhs = xu[:, 0, s] if rr is None else xc[:, rr - 1, s]
                                nc.tensor.matmul(tgt, wt[:, :], rhs,
                                                 start=False, stop=(i == len(xw) - 1))
                        # silu: pt[:, h*512 + k] -> ob[:, 2*pair + h, 512g + k]
                        in_ap = pt[:, :].rearrange("p (h k) -> p h k", h=2)
                        nc.scalar.activation(
                            out=ob[:, 2 * pair : 2 * pair + 2, s],
                            in_=in_ap, func=AF.Silu)
                    # stream this phase-pair out while the next pair computes
                    nc.sync.dma_start(
                        out=out[:, b, 2 * pair : 2 * pair + 2, :],
                        in_=ob[:, 2 * pair : 2 * pair + 2, :])

    nc.compile()
    return nc


_CACHE: dict = {}


def _get_nc():
    if "nc" not in _CACHE:
        _CACHE["nc"] = build_bass()
    return _CACHE["nc"]


def make_in_maps(inputs):
    x = np.asarray(inputs["x"], np.float32)
    delta = np.asarray(inputs["delta"], np.float32).reshape(EMBED_DIM, NDIM)
    alpha = np.asarray(inputs["alpha"], np.float32).reshape(EMBED_DIM, NDIM)
    beta = np.asarray(inputs["beta"], np.float32).reshape(EMBED_DIM, NDIM)
    gamma = np.asarray(inputs["gamma"], np.float32).reshape(EMBED_DIM, NDIM)
    omega = np.asarray(inputs["omega"], np.float32).reshape(EMBED_DIM, 1)
    coef_full = np.concatenate([delta, alpha, beta, gamma, omega], axis=1)
    eye = np.eye(D_PER, dtype=ml_dtypes.bfloat16)
    in_maps = []
    for c in range(N_CORES):
        sl = slice(c * D_PER, (c + 1) * D_PER)
        xc = x[:, :, sl].transpose(2, 1, 0).astype(ml_dtypes.bfloat16)  # [128,B,L]
        ph = xc.reshape(D_PER, BSZ, J, DEC).transpose(0, 1, 3, 2)  # [128,B,4,J]
        xph = np.zeros((D_PER, BSZ, NBLK, J), dtype=ml_dtypes.bfloat16)
        xph[:, :, 0] = ph[:, :, 0]                    # x[4j]
        for k in range(1, 4):                         # x[4(j-1) + (4-k)] = x[4j-k]
            xph[:, :, k, 1:] = ph[:, :, 4 - k, :-1]
        xph[:, :, 4:7] = ph[:, :, 1:4]                # x[4j+r], r=1..3
        in_maps.append(
            {"x": np.ascontiguousarray(xph),
             "coef": np.ascontiguousarray(coef_full[sl]), "eye": eye}
        )
    return in_maps


def gather_out(results):
    out = np.empty((SEQ_LEN, BSZ, EMBED_DIM), np.float32)
    for c in range(N_CORES):
        # [128, B, 4, J] phase-major -> [l = 4j+r, b, d]
        arr = results[c]["out"].astype(np.float32)
        out[:, :, c * D_PER : (c + 1) * D_PER] = arr.transpose(3, 2, 1, 0).reshape(
            SEQ_LEN, BSZ, D_PER
        )
    return out


def _run(inputs, **kwargs):
    nc = _get_nc()
    in_maps = make_in_maps(inputs)
    res = run_bass_kernel_spmd(nc, in_maps, core_ids=list(range(N_CORES)), **kwargs)
    return gather_out(res.results), res


def kernel(**inputs) -> np.ndarray:
    out, _ = _run(inputs)
    return out


# revision 22
# speedup vs baseline: 1.1861x; 1.0181x over previous
"""MultiHeadEMA on 8 Trainium2 NeuronCores.

Strategy
--------
Channel-sharded: embed_dim=1024 -> 8 slices of 128 channels (= SBUF
partitions), one per core. The reference's FFT conv is exactly an order-2 IIR
    y_n[l] = q_n y_n[l-1] + x[l],   out = silu(c0 y0 + c1 y1 + omega x)
computed with `tensor_tensor_scan` on the vector engine.

The DVE scan runs at ~2.1 cyc/elem, so the recurrence is decimated by 4:
    Y_n[j] = y_n[4j] satisfies  Y_n[j] = q_n^4 Y_n[j-1] + u_n[j]
    u_n[j] = x[4j] + q_n x[4j-1] + q_n^2 x[4j-2] + q_n^3 x[4j-3]
u_n is built by accumulating diagonal matmuls (tensor engine, bf16) into
PSUM from contiguous phase blocks of x (deinterleaved and pre-shifted on the
host — a strided matmul rhs halves PE throughput). The scan reads u straight
from PSUM at 1/4 length. Phases y[4j+r] are never materialized: the outputs
    pre_r = c0 y0[4j+r] + c1 y1[4j+r] + w x[4j+r]
expand into diagonal matmuls over (Y0, Y1, phase blocks of x) with
per-channel coefficients (c_n q_n^r, sums), accumulated in PSUM, then one
Silu per 1024 columns evacuates PSUM -> SBUF (phase-major output, host
re-interleaves). Interior is bf16 (fp32 PSUM accumulation, fp32 scan state,
exact fp32 decay factors).

Engine balance at 8 concurrent cores: the chip power governor caps matmuls
at ~379ns (vs 216ns single-core) and punishes load added to the vector
engine (scans degrade), so the design keeps DVE scan-only and feeds the
tensor engine one dense contiguous stream.
"""

import numpy as np
import ml_dtypes

import concourse.bass as bass
import concourse.bacc as bacc
import concourse.tile as tile
from concourse import mybir
from concourse.bass_utils import run_bass_kernel_spmd

SEQ_LEN, BSZ, EMBED_DIM, NDIM = 4096, 4, 1024, 2
N_CORES = 8
D_PER = EMBED_DIM // N_CORES  # 128 channels/core = full SBUF partitions
SCALE = (1.0 / NDIM) ** 0.5
DEC = 4                   # decimation factor
J = SEQ_LEN // DEC        # decimated length 1024
CH = 512                  # matmul chunk (one fp32 PSUM bank)
NG = J // CH              # j-groups per slab (2)
F32 = mybir.dt.float32
BF16 = mybir.dt.bfloat16
AF = mybir.ActivationFunctionType
ALU = mybir.AluOpType

# x phase blocks, ordered so the u-generation inputs come first:
#   block 0..3: x[4j], x[4(j-1)+3], x[4(j-1)+2], x[4(j-1)+1]   (u inputs)
#   block 4..6: x[4j+1], x[4j+2], x[4j+3]                      (combine inputs)
NBLK = 7


def build_bass():
    nc = bacc.Bacc(name="multihead_ema")
    x = nc.dram_tensor("x", [D_PER, BSZ, NBLK, J], BF16, kind="ExternalInput")
    # coef columns: [delta0, delta1, alpha0, alpha1, beta0, beta1, gamma0, gamma1, omega]
    coef = nc.dram_tensor("coef", [D_PER, 9], F32, kind="ExternalInput")
    eye = nc.dram_tensor("eye", [D_PER, D_PER], BF16, kind="ExternalInput")
    out = nc.dram_tensor("out", [D_PER, BSZ, DEC, J], BF16, kind="ExternalOutput")

    with tile.TileContext(nc) as tc:
        with (
            tc.tile_pool(name="const", bufs=1) as const,
            tc.tile_pool(name="xup", bufs=4) as xup,
            tc.tile_pool(name="xcp", bufs=4) as xcp,
            tc.tile_pool(name="yp", bufs=2) as yp,
            tc.tile_pool(name="op", bufs=3) as op,
            tc.tile_pool(name="psu", bufs=2, space="PSUM") as psu,
            tc.tile_pool(name="psc", bufs=2, space="PSUM") as psc,
        ):
            csb = const.tile([D_PER, 9], F32)
            nc.sync.dma_start(out=csb[:, :], in_=coef[:, :])
            eyesb = const.tile([D_PER, D_PER], BF16)
            nc.sync.dma_start(out=eyesb[:, :], in_=eye[:, :])

            # --- per-channel coefficients ([128, 1/2] fp32, trivial)
            sig = const.tile([D_PER, 4], F32)  # [p0, p1, sa0, sa1]
            nc.scalar.activation(out=sig[:, :], in_=csb[:, 0:4], func=AF.Sigmoid)
            pq = const.tile([D_PER, NDIM], F32)
            nc.vector.tensor_mul(out=pq[:, :], in0=sig[:, 0:2], in1=sig[:, 2:4])
            q = const.tile([D_PER, NDIM], F32)  # q = 1 - p*sigmoid(alpha)
            nc.scalar.activation(out=q[:, :], in_=pq[:, :], func=AF.Copy,
                                 scale=-1.0, bias=1.0)
            q2 = const.tile([D_PER, NDIM], F32)
            nc.vector.tensor_mul(out=q2[:, :], in0=q[:, :], in1=q[:, :])
            q3 = const.tile([D_PER, NDIM], F32)
            nc.vector.tensor_mul(out=q3[:, :], in0=q2[:, :], in1=q[:, :])
            q4 = const.tile([D_PER, NDIM], F32)
            nc.vector.tensor_mul(out=q4[:, :], in0=q2[:, :], in1=q2[:, :])
            c1t = const.tile([D_PER, NDIM], F32)
            nc.vector.tensor_mul(out=c1t[:, :], in0=sig[:, 0:2], in1=csb[:, 4:6])
            c2t = const.tile([D_PER, NDIM], F32)
            nc.vector.tensor_mul(out=c2t[:, :], in0=c1t[:, :], in1=csb[:, 6:8])
            cc = const.tile([D_PER, NDIM], F32)  # c_n = p beta gamma scale
            nc.scalar.mul(out=cc[:, :], in_=c2t[:, :], mul=SCALE)
            cq = const.tile([D_PER, NDIM], F32)   # c_n q_n
            nc.vector.tensor_mul(out=cq[:, :], in0=cc[:, :], in1=q[:, :])
            cq2 = const.tile([D_PER, NDIM], F32)  # c_n q_n^2
            nc.vector.tensor_mul(out=cq2[:, :], in0=cc[:, :], in1=q2[:, :])
            cq3 = const.tile([D_PER, NDIM], F32)  # c_n q_n^3
            nc.vector.tensor_mul(out=cq3[:, :], in0=cc[:, :], in1=q3[:, :])
            csum = const.tile([D_PER, 1], F32)    # c0 + c1 + w
            nc.vector.tensor_add(out=csum[:, :], in0=cc[:, 0:1], in1=cc[:, 1:2])
            nc.vector.tensor_add(out=csum[:, :], in0=csum[:, :], in1=csb[:, 8:9])
            cqs = const.tile([D_PER, 1], F32)     # c0 q0 + c1 q1
            nc.vector.tensor_add(out=cqs[:, :], in0=cq[:, 0:1], in1=cq[:, 1:2])
            cq2s = const.tile([D_PER, 1], F32)    # c0 q0^2 + c1 q1^2
            nc.vector.tensor_add(out=cq2s[:, :], in0=cq2[:, 0:1], in1=cq2[:, 1:2])

            # --- bf16 diagonal weight matrices
            _dn = [0]

            def diag(scalar_ap):
                _dn[0] += 1
                t = const.tile([D_PER, D_PER], BF16, tag=f"diag{_dn[0]}")
                nc.vector.tensor_scalar_mul(out=t[:, :], in0=eyesb[:, :],
                                            scalar1=scalar_ap)
                return t

            w_q = [[diag(t[:, n : n + 1]) for n in range(NDIM)] for t in (q, q2, q3)]
            w_cy = [[diag(t[:, n : n + 1]) for n in range(NDIM)]
                    for t in (cc, cq, cq2, cq3)]  # Y-term weights for r=0..3
            w_w = diag(csb[:, 8:9])    # x term of pre_0
            w_cw = diag(csum[:, 0:1])  # x_pr self term, r>=1
            w_cqs = diag(cqs[:, 0:1])
            w_cq2s = diag(cq2s[:, 0:1])

            q4b = [q4[:, n : n + 1].to_broadcast([D_PER, J]) for n in range(NDIM)]

            for b in range(BSZ):
                xall = xup.tile([D_PER, NBLK, J], BF16)
                nc.sync.dma_start(out=xall[:, :, :], in_=x[:, b, :, :])
                xu = xall[:, 0:4, :]
                xc = xall[:, 4:7, :]

                # --- u_n in PSUM, Y_n = scan(q_n^4, u_n)
                Y = []
                for n in range(NDIM):
                    pu = psu.tile([D_PER, J], F32, tag="u")
                    for g in range(NG):
                        s = bass.ts(g, CH)
                        nc.tensor.matmul(pu[:, s], eyesb[:, :], xu[:, 0, s],
                                         start=True, stop=False)
                        for k in range(1, 4):  # + q^k * x[4j-k] (pre-shifted block k)
                            nc.tensor.matmul(pu[:, s], w_q[k - 1][n][:, :],
                                             xu[:, k, s],
                                             start=False, stop=(k == 3))
                    yn = yp.tile([D_PER, J], BF16, tag=f"y{n}")
                    nc.vector.tensor_tensor_scan(
                        out=yn[:, :], data0=q4b[n], data1=pu[:, :],
                        initial=0.0, op0=ALU.mult, op1=ALU.add,
                    )
                    Y.append(yn)

                # --- outputs: pre_r accumulated in PSUM, silu evacuates
                ob = op.tile([D_PER, DEC, J], BF16)
                for pair in range(2):  # phases (0,1) then (2,3)
                    for g in range(NG):
                        s = bass.ts(g, CH)
                        pt = psc.tile([D_PER, 2 * CH], F32, tag="cmb")
                        for h in range(2):
                            r = 2 * pair + h
                            tgt = pt[:, bass.ts(h, CH)]
                            nc.tensor.matmul(tgt, w_cy[r][0][:, :], Y[0][:, s],
                                             start=True, stop=False)
                            nc.tensor.matmul(tgt, w_cy[r][1][:, :], Y[1][:, s],
                                             start=False, stop=False)
                            # x terms: phase r block is xc[r-1] (r>=1), xu[0] for r=0
                            xw = [(w_w, None) if r == 0 else (w_cw, r)]
                            if r == 2:
                                xw.append((w_cqs, 1))
                            elif r == 3:
                                xw.append((w_cqs, 2))
                                xw.append((w_cq2s, 1))
                            for i, (wt, rr) in enumerate(xw):
                                rhs = xu[:, 0, s] if rr is None else xc[:, rr - 1, s]
                                nc.tensor.matmul(tgt, wt[:, :], rhs,
                                                 start=False, stop=(i == len(xw) - 1))
                        # silu: pt[:, h*512 + k] -> ob[:, 2*pair + h, 512g + k]
                        in_ap = pt[:, :].rearrange("p (h k) -> p h k", h=2)
                        nc.scalar.activation(
                            out=ob[:, 2 * pair : 2 * pair + 2, s],
                            in_=in_ap, func=AF.Silu)
                    # stream this phase-pair out while the next pair computes
                    nc.sync.dma_start(
                        out=out[:, b, 2 * pair : 2 * pair + 2, :],
                        in_=ob[:, 2 * pair : 2 * pair + 2, :])

    nc.compile()
    return nc


_CACHE: dict = {}


def _get_nc():
    if "nc" not in _CACHE:
        _CACHE["nc"] = build_bass()
    return _CACHE["nc"]


def make_in_maps(inputs):
    x = np.asarray(inputs["x"], np.float32)
    delta = np.asarray(inputs["delta"], np.float32).reshape(EMBED_DIM, NDIM)
    alpha = np.asarray(inputs["alpha"], np.float32).reshape(EMBED_DIM, NDIM)
    beta = np.asarray(inputs["beta"], np.float32).reshape(EMBED_DIM, NDIM)
    gamma = np.asarray(inputs["gamma"], np.float32).reshape(EMBED_DIM, NDIM)
    omega = np.asarray(inputs["omega"], np.float32).reshape(EMBED_DIM, 1)
    coef_full = np.concatenate([delta, alpha, beta, gamma, omega], axis=1)
    eye = np.eye(D_PER, dtype=ml_dtypes.bfloat16)
    in_maps = []
    for c in range(N_CORES):
        sl = slice(c * D_PER, (c + 1) * D_PER)
        xc = x[:, :, sl].transpose(2, 1, 0).astype(ml_dtypes.bfloat16)  # [128,B,L]
        ph = xc.reshape(D_PER, BSZ, J, DEC).transpose(0, 1, 3, 2)  # [128,B,4,J]
        xph = np.zeros((D_PER, BSZ, NBLK, J), dtype=ml_dtypes.bfloat16)
        xph[:, :, 0] = ph[:, :, 0]                    # x[4j]
        for k in range(1, 4):                         # x[4(j-1) + (4-k)] = x[4j-k]
            xph[:, :, k, 1:] = ph[:, :, 4 - k, :-1]
        xph[:, :, 4:7] = ph[:, :, 1:4]                # x[4j+r], r=1..3
        in_maps.append(
            {"x": np.ascontiguousarray(xph),
             "coef": np.ascontiguousarray(coef_full[sl]), "eye": eye}
        )
    return in_maps


def gather_out(results):
    out = np.empty((SEQ_LEN, BSZ, EMBED_DIM), np.float32)
    for c in range(N_CORES):
        # [128, B, 4, J] phase-major -> [l = 4j+r, b, d]
        arr = results[c]["out"].astype(np.float32)
        out[:, :, c * D_PER : (c + 1) * D_PER] = arr.transpose(3, 2, 1, 0).reshape(
            SEQ_LEN, BSZ, D_PER
        )
    return out


def _run(inputs, **kwargs):
    nc = _get_nc()
    in_maps = make_in_maps(inputs)
    res = run_bass_kernel_spmd(nc, in_maps, core_ids=list(range(N_CORES)), **kwargs)
    return gather_out(res.results), res


def kernel(**inputs) -> np.ndarray:
    out, _ = _run(inputs)
    return out


# revision 23
# speedup vs baseline: 1.2093x; 1.0196x over previous
"""MultiHeadEMA on 8 Trainium2 NeuronCores.

Strategy
--------
Channel-sharded: embed_dim=1024 -> 8 slices of 128 channels (= SBUF
partitions), one per core. The reference's FFT conv is exactly an order-2 IIR
    y_n[l] = q_n y_n[l-1] + x[l],   out = silu(c0 y0 + c1 y1 + omega x)
computed with `tensor_tensor_scan` on the vector engine.

The DVE scan runs at ~2.1 cyc/elem, so the recurrence is decimated by 4:
    Y_n[j] = y_n[4j] satisfies  Y_n[j] = q_n^4 Y_n[j-1] + u_n[j]
    u_n[j] = x[4j] + q_n x[4j-1] + q_n^2 x[4j-2] + q_n^3 x[4j-3]
u_n is built by accumulating diagonal matmuls (tensor engine, bf16) into
PSUM from contiguous phase blocks of x (deinterleaved and pre-shifted on the
host — a strided matmul rhs halves PE throughput). The scan reads u straight
from PSUM at 1/4 length. Phases y[4j+r] are never materialized: the outputs
    pre_r = c0 y0[4j+r] + c1 y1[4j+r] + w x[4j+r]
expand into diagonal matmuls over (Y0, Y1, phase blocks of x) with
per-channel coefficients (c_n q_n^r, sums), accumulated in PSUM, then one
Silu per 1024 columns evacuates PSUM -> SBUF (phase-major output, host
re-interleaves). Interior is bf16 (fp32 PSUM accumulation, fp32 scan state,
exact fp32 decay factors).

Engine balance at 8 concurrent cores: the chip power governor caps matmuls
at ~379ns (vs 216ns single-core) and punishes load added to the vector
engine (scans degrade), so the design keeps DVE scan-only and feeds the
tensor engine one dense contiguous stream.
"""

import numpy as np
import ml_dtypes

import concourse.bass as bass
import concourse.bacc as bacc
import concourse.tile as tile
from concourse import mybir
from concourse.bass_utils import run_bass_kernel_spmd

SEQ_LEN, BSZ, EMBED_DIM, NDIM = 4096, 4, 1024, 2
N_CORES = 8
D_PER = EMBED_DIM // N_CORES  # 128 channels/core = full SBUF partitions
SCALE = (1.0 / NDIM) ** 0.5
DEC = 4                   # decimation factor
J = SEQ_LEN // DEC        # decimated length 1024
CH = 512                  # matmul chunk (one fp32 PSUM bank)
NG = J // CH              # j-groups per slab (2)
F32 = mybir.dt.float32
BF16 = mybir.dt.bfloat16
AF = mybir.ActivationFunctionType
ALU = mybir.AluOpType

# x phase blocks, ordered so the u-generation inputs come first:
#   block 0..3: x[4j], x[4(j-1)+3], x[4(j-1)+2], x[4(j-1)+1]   (u inputs)
#   block 4..6: x[4j+1], x[4j+2], x[4j+3]                      (combine inputs)
NBLK = 7


def build_bass():
    nc = bacc.Bacc(name="multihead_ema")
    x = nc.dram_tensor("x", [D_PER, BSZ, NBLK, J], BF16, kind="ExternalInput")
    # coef columns: [delta0, delta1, alpha0, alpha1, beta0, beta1, gamma0, gamma1, omega]
    coef = nc.dram_tensor("coef", [D_PER, 9], F32, kind="ExternalInput")
    eye = nc.dram_tensor("eye", [D_PER, D_PER], BF16, kind="ExternalInput")
    out = nc.dram_tensor("out", [D_PER, BSZ, DEC, J], BF16, kind="ExternalOutput")

    with tile.TileContext(nc) as tc:
        with (
            tc.tile_pool(name="const", bufs=1) as const,
            tc.tile_pool(name="xup", bufs=4) as xup,
            tc.tile_pool(name="xcp", bufs=4) as xcp,
            tc.tile_pool(name="yp", bufs=2) as yp,
            tc.tile_pool(name="op", bufs=3) as op,
            tc.tile_pool(name="psu", bufs=2, space="PSUM") as psu,
            tc.tile_pool(name="psc", bufs=2, space="PSUM") as psc,
        ):
            csb = const.tile([D_PER, 9], F32)
            nc.sync.dma_start(out=csb[:, :], in_=coef[:, :])
            eyesb = const.tile([D_PER, D_PER], BF16)
            nc.sync.dma_start(out=eyesb[:, :], in_=eye[:, :])

            # --- per-channel coefficients ([128, 1/2] fp32, trivial)
            sig = const.tile([D_PER, 4], F32)  # [p0, p1, sa0, sa1]
            nc.scalar.activation(out=sig[:, :], in_=csb[:, 0:4], func=AF.Sigmoid)
            pq = const.tile([D_PER, NDIM], F32)
            nc.vector.tensor_mul(out=pq[:, :], in0=sig[:, 0:2], in1=sig[:, 2:4])
            q = const.tile([D_PER, NDIM], F32)  # q = 1 - p*sigmoid(alpha)
            nc.scalar.activation(out=q[:, :], in_=pq[:, :], func=AF.Copy,
                                 scale=-1.0, bias=1.0)
            q2 = const.tile([D_PER, NDIM], F32)
            nc.vector.tensor_mul(out=q2[:, :], in0=q[:, :], in1=q[:, :])
            q3 = const.tile([D_PER, NDIM], F32)
            nc.vector.tensor_mul(out=q3[:, :], in0=q2[:, :], in1=q[:, :])
            q4 = const.tile([D_PER, NDIM], F32)
            nc.vector.tensor_mul(out=q4[:, :], in0=q2[:, :], in1=q2[:, :])
            c1t = const.tile([D_PER, NDIM], F32)
            nc.vector.tensor_mul(out=c1t[:, :], in0=sig[:, 0:2], in1=csb[:, 4:6])
            c2t = const.tile([D_PER, NDIM], F32)
            nc.vector.tensor_mul(out=c2t[:, :], in0=c1t[:, :], in1=csb[:, 6:8])
            cc = const.tile([D_PER, NDIM], F32)  # c_n = p beta gamma scale
            nc.scalar.mul(out=cc[:, :], in_=c2t[:, :], mul=SCALE)
            cq = const.tile([D_PER, NDIM], F32)   # c_n q_n
            nc.vector.tensor_mul(out=cq[:, :], in0=cc[:, :], in1=q[:, :])
            cq2 = const.tile([D_PER, NDIM], F32)  # c_n q_n^2
            nc.vector.tensor_mul(out=cq2[:, :], in0=cc[:, :], in1=q2[:, :])
            cq3 = const.tile([D_PER, NDIM], F32)  # c_n q_n^3
            nc.vector.tensor_mul(out=cq3[:, :], in0=cc[:, :], in1=q3[:, :])
            csum = const.tile([D_PER, 1], F32)    # c0 + c1 + w
            nc.vector.tensor_add(out=csum[:, :], in0=cc[:, 0:1], in1=cc[:, 1:2])
            nc.vector.tensor_add(out=csum[:, :], in0=csum[:, :], in1=csb[:, 8:9])
            cqs = const.tile([D_PER, 1], F32)     # c0 q0 + c1 q1
            nc.vector.tensor_add(out=cqs[:, :], in0=cq[:, 0:1], in1=cq[:, 1:2])
            cq2s = const.tile([D_PER, 1], F32)    # c0 q0^2 + c1 q1^2
            nc.vector.tensor_add(out=cq2s[:, :], in0=cq2[:, 0:1], in1=cq2[:, 1:2])

            # --- bf16 diagonal weight matrices
            _dn = [0]

            def diag(scalar_ap):
                _dn[0] += 1
                t = const.tile([D_PER, D_PER], BF16, tag=f"diag{_dn[0]}")
                nc.vector.tensor_scalar_mul(out=t[:, :], in0=eyesb[:, :],
                                            scalar1=scalar_ap)
                return t

            w_q = [[diag(t[:, n : n + 1]) for n in range(NDIM)] for t in (q, q2, q3)]
            w_cy = [[diag(t[:, n : n + 1]) for n in range(NDIM)]
                    for t in (cc, cq, cq2, cq3)]  # Y-term weights for r=0..3
            w_w = diag(csb[:, 8:9])    # x term of pre_0
            w_cw = diag(csum[:, 0:1])  # x_pr self term, r>=1
            w_cqs = diag(cqs[:, 0:1])
            w_cq2s = diag(cq2s[:, 0:1])

            q4b = [q4[:, n : n + 1].to_broadcast([D_PER, J]) for n in range(NDIM)]

            # prefetch all slabs; u-blocks in their own (earlier) transfers so
            # the first matmuls are gated by a 1MB DMA instead of 1.75MB
            xus, xcs = [], []
            for b in range(BSZ):
                xu = xup.tile([D_PER, 4, J], BF16, tag="xu")
                nc.sync.dma_start(out=xu[:, :, :], in_=x[:, b, 0:4, :])
                xc = xcp.tile([D_PER, 3, J], BF16, tag="xc")
                nc.sync.dma_start(out=xc[:, :, :], in_=x[:, b, 4:7, :])
                xus.append(xu)
                xcs.append(xc)

            for b in range(BSZ):
                xu = xus[b]
                xc = xcs[b]

                # --- u_n in PSUM, Y_n = scan(q_n^4, u_n)
                Y = []
                for n in range(NDIM):
                    pu = psu.tile([D_PER, J], F32, tag="u")
                    for g in range(NG):
                        s = bass.ts(g, CH)
                        nc.tensor.matmul(pu[:, s], eyesb[:, :], xu[:, 0, s],
                                         start=True, stop=False)
                        for k in range(1, 4):  # + q^k * x[4j-k] (pre-shifted block k)
                            nc.tensor.matmul(pu[:, s], w_q[k - 1][n][:, :],
                                             xu[:, k, s],
                                             start=False, stop=(k == 3))
                    yn = yp.tile([D_PER, J], BF16, tag=f"y{n}")
                    nc.vector.tensor_tensor_scan(
                        out=yn[:, :], data0=q4b[n], data1=pu[:, :],
                        initial=0.0, op0=ALU.mult, op1=ALU.add,
                    )
                    Y.append(yn)

                # --- outputs: pre_r accumulated in PSUM, silu evacuates
                ob = op.tile([D_PER, DEC, J], BF16)
                for pair in range(2):  # phases (0,1) then (2,3)
                    for g in range(NG):
                        s = bass.ts(g, CH)
                        pt = psc.tile([D_PER, 2 * CH], F32, tag="cmb")
                        for h in range(2):
                            r = 2 * pair + h
                            tgt = pt[:, bass.ts(h, CH)]
                            nc.tensor.matmul(tgt, w_cy[r][0][:, :], Y[0][:, s],
                                             start=True, stop=False)
                            nc.tensor.matmul(tgt, w_cy[r][1][:, :], Y[1][:, s],
                                             start=False, stop=False)
                            # x terms: phase r block is xc[r-1] (r>=1), xu[0] for r=0
                            xw = [(w_w, None) if r == 0 else (w_cw, r)]
                            if r == 2:
                                xw.append((w_cqs, 1))
                            elif r == 3:
                                xw.append((w_cqs, 2))
                                xw.append((w_cq2s, 1))
                            for i, (wt, rr) in enumerate(xw):
                                rhs = xu[:, 0, s] if rr is None else xc[:, rr - 1, s]
                                nc.tensor.matmul(tgt, wt[:, :], rhs,
                                                 start=False, stop=(i == len(xw) - 1))
                        # silu: pt[:, h*512 + k] -> ob[:, 2*pair + h, 512g + k]
                        in_ap = pt[:, :].rearrange("p (h k) -> p h k", h=2)
                        nc.scalar.activation(
                            out=ob[:, 2 * pair : 2 * pair + 2, s],
                            in_=in_ap, func=AF.Silu)
                    # stream this phase-pair out while the next pair computes
                    nc.sync.dma_start(
                        out=out[:, b, 2 * pair : 2 * pair + 2, :],
                        in_=ob[:, 2 * pair : 2 * pair + 2, :])

    nc.compile()
    return nc


_CACHE: dict = {}


def _get_nc():
    if "nc" not in _CACHE:
        _CACHE["nc"] = build_bass()
    return _CACHE["nc"]


def make_in_maps(inputs):
    x = np.asarray(inputs["x"], np.float32)
    delta = np.asarray(inputs["delta"], np.float32).reshape(EMBED_DIM, NDIM)
    alpha = np.asarray(inputs["alpha"], np.float32).reshape(EMBED_DIM, NDIM)
    beta = np.asarray(inputs["beta"], np.float32).reshape(EMBED_DIM, NDIM)
    gamma = np.asarray(inputs["gamma"], np.float32).reshape(EMBED_DIM, NDIM)
    omega = np.asarray(inputs["omega"], np.float32).reshape(EMBED_DIM, 1)
    coef_full = np.concatenate([delta, alpha, beta, gamma, omega], axis=1)
    eye = np.eye(D_PER, dtype=ml_dtypes.bfloat16)
    in_maps = []
    for c in range(N_CORES):
        sl = slice(c * D_PER, (c + 1) * D_PER)
        xc = x[:, :, sl].transpose(2, 1, 0).astype(ml_dtypes.bfloat16)  # [128,B,L]
        ph = xc.reshape(D_PER, BSZ, J, DEC).transpose(0, 1, 3, 2)  # [128,B,4,J]
        xph = np.zeros((D_PER, BSZ, NBLK, J), dtype=ml_dtypes.bfloat16)
        xph[:, :, 0] = ph[:, :, 0]                    # x[4j]
        for k in range(1, 4):                         # x[4(j-1) + (4-k)] = x[4j-k]
            xph[:, :, k, 1:] = ph[:, :, 4 - k, :-1]
        xph[:, :, 4:7] = ph[:, :, 1:4]                # x[4j+r], r=1..3
        in_maps.append(
            {"x": np.ascontiguousarray(xph),
             "coef": np.ascontiguousarray(coef_full[sl]), "eye": eye}
        )
    return in_maps


def gather_out(results):
    out = np.empty((SEQ_LEN, BSZ, EMBED_DIM), np.float32)
    for c in range(N_CORES):
        # [128, B, 4, J] phase-major -> [l = 4j+r, b, d]
        arr = results[c]["out"].astype(np.float32)
        out[:, :, c * D_PER : (c + 1) * D_PER] = arr.transpose(3, 2, 1, 0).reshape(
            SEQ_LEN, BSZ, D_PER
        )
    return out


def _run(inputs, **kwargs):
    nc = _get_nc()
    in_maps = make_in_maps(inputs)
    res = run_bass_kernel_spmd(nc, in_maps, core_ids=list(range(N_CORES)), **kwargs)
    return gather_out(res.results), res


def kernel(**inputs) -> np.ndarray:
    out, _ = _run(inputs)
    return out
